# revision 9
# baseline (speedup 1.0000x reference)
"""DynamicUpsamplingFilter Bass/Trainium2 kernel.

out[b, c*16+r, h, w] = sum_{di,dj} x_pad[b, c, h+di, w+dj] * filters[b, di*5+dj, r, h, w]

Sharding: pure data parallel, one batch element per NeuronCore (B=8, 8 cores).

Per-core device layout:
  partition dim = (h_sub, w_half) -> 128 partitions (3 row blocks: 64/64/52 rows)
  free dim     = (r=16, w=160)
For each of the 25 taps: load filt[f] tile (fp32, HWDGE), cast to fp16 on ACT,
then DVE tensor_mul with the shifted x window broadcast over r (zero-stride AP),
accumulating in fp16 with grouped adds (5 groups of 5 taps).
The padded x is precomputed on host (fp16, [3,184,324]) — two copies shifted by
one element so that every tap's innermost AP offset stays 4-byte aligned.
"""

import numpy as np

import concourse.bass as bass
import concourse.bacc as bacc
import concourse.mybir as mybir
from concourse.tile import TileContext
from concourse.bass_utils import run_bass_kernel_spmd

B, C, H, W = 8, 3, 180, 320
NF, R = 25, 16
K, PAD = 5, 2
HP, WP = H + 2 * PAD, W + 2 * PAD  # 184, 324
WH = W // 2  # 160, w-half size

DT = mybir.dt.float16

HBLOCKS = [(0, 64), (64, 64), (128, 52)]

_CACHED = {}


def _build_nc():
    nc = bacc.Bacc("TRN2", target_bir_lowering=False, debug=False, num_devices=8)
    # xv: host-precomputed padded x windows in fp16, already laid out exactly
    # like the device tiles: [c, parity, partition(h,wh across 3 hblocks), di, u].
    # parity 1 is shifted one element left so odd-dj taps stay 4B-aligned.
    TP = sum(hs * 2 for _, hs in HBLOCKS)  # total partitions across hblocks
    xv = nc.dram_tensor("xv", [C, 2, TP, K, WH + 4], DT, kind="ExternalInput")
    filt = nc.dram_tensor("filt", [NF, R, H, W], mybir.dt.float32, kind="ExternalInput")
    out = nc.dram_tensor("out", [C * R, H, W], mybir.dt.float32, kind="ExternalOutput")

    hblocks = HBLOCKS

    with TileContext(nc) as tc:
        with tc.tile_pool(name="p", bufs=1) as pool:
            pstart = 0
            for hb0, hs in hblocks:
                parts = hs * 2
                # x window tiles: [parts, K(di), WH+4] per channel & parity
                xt = []
                for c in range(C):
                    pair = []
                    for par in range(2):
                        t = pool.tile([128, K, WH + 4], DT, tag=f"x{c}{par}", bufs=2, name=f"xt{c}{par}")
                        nc.sync.dma_start(
                            out=t[:parts], in_=xv[c, par, pstart : pstart + parts]
                        )
                        pair.append(t)
                    xt.append(pair)
                pstart += parts

                # accumulation: group 0 (taps 0-4) accumulates directly into
                # acc[c]; groups 1-4 build a 5-tap group sum then fold it in.
                accs = [
                    pool.tile([128, R, WH], DT, tag=f"a{c}", bufs=2, name=f"acc{c}") for c in range(C)
                ]
                gaccs = [None] * C  # current 5-tap group accumulator
                for f in range(NF):
                    di, dj = f // K, f % K
                    ft32 = pool.tile([128, R, WH], mybir.dt.float32, tag="f32", bufs=3, name="ft32")
                    src = filt[f, :, hb0 : hb0 + hs, :].rearrange(
                        "r h (wh w) -> (h wh) r w", wh=2
                    )
                    nc.sync.dma_start(out=ft32[:parts], in_=src)
                    ft16 = pool.tile([128, R, WH], DT, tag="f16", bufs=3, name="ft16")
                    nc.scalar.copy(out=ft16[:parts], in_=ft32[:parts])

                    for c in range(C):
                        par = dj % 2
                        off = dj - par
                        xin = (
                            xt[c][par][:parts, di, off : off + WH]
                            .unsqueeze(1)
                            .broadcast_to([parts, R, WH])
                        )
                        first_of_group = f % K == 0
                        tgt = accs[c] if f < K else gaccs[c]
                        if f == 0 or (first_of_group and f >= K):
                            if first_of_group and f >= K:
                                tgt = pool.tile([128, R, WH], DT, tag=f"g{c}", bufs=2, name=f"g{c}")
                                gaccs[c] = tgt
                            nc.vector.tensor_mul(
                                out=tgt[:parts], in0=ft16[:parts], in1=xin
                            )
                        else:
                            p = pool.tile([128, R, WH], DT, tag="prod", bufs=3, name="prod")
                            nc.vector.tensor_mul(out=p[:parts], in0=ft16[:parts], in1=xin)
                            nc.vector.tensor_add(
                                out=tgt[:parts], in0=tgt[:parts], in1=p[:parts]
                            )
                        if f % K == K - 1 and f >= K:  # fold group into acc
                            nc.vector.tensor_add(
                                out=accs[c][:parts],
                                in0=accs[c][:parts],
                                in1=gaccs[c][:parts],
                            )

                for c in range(C):
                    o32 = pool.tile([128, R, WH], mybir.dt.float32, tag="o32", bufs=2, name="o32")
                    nc.scalar.copy(out=o32[:parts], in_=accs[c][:parts])
                    dst = out[c * R : (c + 1) * R, hb0 : hb0 + hs, :].rearrange(
                        "r h (wh w) -> (h wh) r w", wh=2
                    )
                    nc.sync.dma_start(out=dst, in_=o32[:parts])

    nc.compile()
    return nc


def _get_nc():
    if "nc" not in _CACHED:
        _CACHED["nc"] = _build_nc()
    return _CACHED["nc"]


def _prep_maps(x, filters):
    # padded x, one extra zero column so the parity-1 (shift-by-one) windows
    # at the right edge stay in bounds
    xp = np.zeros((B, C, HP, WP + 1), np.float16)
    xp[:, :, PAD : PAD + H, PAD : PAD + W] = x.astype(np.float16)
    TP = sum(hs * 2 for _, hs in HBLOCKS)
    xv = np.zeros((B, C, 2, TP, K, WH + 4), np.float16)
    for par in range(2):
        pstart = 0
        for hb0, hs in HBLOCKS:
            for di in range(K):
                for wh in range(2):
                    col = wh * WH + par
                    # partition (h, wh) = pstart + 2*h + wh
                    xv[:, :, par, pstart + wh : pstart + 2 * hs : 2, di, :] = xp[
                        :, :, hb0 + di : hb0 + di + hs, col : col + WH + 4
                    ]
            pstart += 2 * hs
    maps = []
    for b in range(B):
        maps.append(
            {
                "xv": xv[b],
                "filt": np.ascontiguousarray(filters[b]),
            }
        )
    return maps


def kernel(x: np.ndarray, filters: np.ndarray, _trace=False, _trace_kwargs=None):
    nc = _get_nc()
    maps = _prep_maps(np.asarray(x), np.asarray(filters))
    kw = {}
    if _trace:
        kw.update(trace=True, **(_trace_kwargs or {}))
    res = run_bass_kernel_spmd(nc, maps, list(range(B)), **kw)
    out = np.stack([res.results[b]["out"] for b in range(B)], axis=0)
    out = out.reshape(B, C * R, H, W).astype(np.float32)
    if _trace:
        return out, res
    return out
